# revision 5
# baseline (speedup 1.0000x reference)
"""Trainium2 Bass kernel for nn_AttentionLayer (sparse_attention).

Computation (per reference):
    xf = x.reshape(B, C, S);  S = W*H = 4096
    q = xf @ Wq.T + bq            [B, C, 16]
    k = xf @ Wk.T + bk            [B, C, 16]
    kq[b] = q[b] @ k[b].T         [B, C, C]
    A = softmax(kq, axis=0)       (over the batch axis -- Softmax2d)
    out[b] = A[b].T @ xf[b]       [B, C, S]

Sharding: data-parallel over batch, 2 batches per core (8 cores).  The
axis-0 softmax couples cores only through the denominator sum(exp) over
b, which is exchanged with a single 1 MiB AllReduce.  exp(kq) needs no
max subtraction: |kq| < ~40 on this distribution, well inside fp32 exp
range.

On-core pipeline per batch:
  1. PE-transpose x tiles (128x128) to get s-major tiles for the q/k
     projections (contraction over s requires s on partitions).
  2. qT/kT = Wqk_T.T @ xT, accumulated over 32 s-chunks in PSUM.
  3. kq = q @ k.T (K=16), exp via ScalarE straight out of PSUM.
  4. S_loc = sum_b_local exp;  AllReduce(add);  A = exp * (1/S).
  5. out[b] = A[b].T @ x[b] as 128x512 PSUM tiles accumulated over the
     4 i-chunks, evacuated via DVE/ACT copies, DMA'd out.

Matmuls use float32r views of the fp32 data (full-rate on the PE at
moving-dim >= 256, ~tf32-class precision).
"""

import os
import numpy as np

import concourse.mybir as mybir
import concourse.tile as tile
from concourse import bacc
from concourse.bass_utils import run_bass_kernel_spmd
from concourse.masks import make_identity

B, C, S, D = 16, 512, 4096, 16
N_CORES = 8
B_LOC = B // N_CORES          # 2 batches per core
CC = C // 128                 # 4 c-chunks
SC = S // 128                 # 32 s-chunks
F32 = mybir.dt.float32
F32R = mybir.dt.float32r

_CACHE = {}


def _build():
    nc = bacc.Bacc("TRN2", target_bir_lowering=False, debug=False,
                   num_devices=N_CORES)
    x_d = nc.dram_tensor("x", [B_LOC, C, S], F32R, kind="ExternalInput")
    w_d = nc.dram_tensor("wqkT", [S, 2 * D], F32R, kind="ExternalInput")
    b_d = nc.dram_tensor("bqk", [2 * D, 1], F32, kind="ExternalInput")
    out_d = nc.dram_tensor("out", [B_LOC, C, S], F32, kind="ExternalOutput")

    with tile.TileContext(nc) as tc:
        with (
            tc.tile_pool(name="persist", bufs=1) as persist,
            tc.tile_pool(name="xt", bufs=3) as xtp,
            tc.tile_pool(name="outsb", bufs=8) as outp,
            tc.tile_pool(name="dram", bufs=1, space="DRAM") as dram,
        ):
            # ---- constants / inputs to SBUF ----
            ident = persist.tile([128, 128], F32, tag="ident", name="ident")
            make_identity(nc, ident)
            wqk = persist.tile([128, SC, 2 * D], F32R, tag="wqk", name="wqk")
            nc.sync.dma_start(
                out=wqk, in_=w_d.ap().rearrange("(n p) d -> p n d", p=128))
            bqk = persist.tile([2 * D, 1], F32, tag="bqk", name="bqk")
            nc.sync.dma_start(out=bqk, in_=b_d.ap())

            x_sb = [[persist.tile([128, S], F32R, tag=f"x{b}_{cc}", name=f"x{b}_{cc}")
                     for cc in range(CC)] for b in range(B_LOC)]
            for b in range(B_LOC):
                for cc in range(CC):
                    nc.sync.dma_start(
                        out=x_sb[b][cc],
                        in_=x_d.ap()[b, cc * 128:(cc + 1) * 128, :])

            qkb_sb = [persist.tile([2 * D, C], F32R, tag=f"qkb{b}", name=f"qkb{b}")
                      for b in range(B_LOC)]
            k_sb = [persist.tile([D, C], F32R, tag=f"k{b}", name=f"k{b}")
                    for b in range(B_LOC)]
            E_sb = [persist.tile([128, CC * C], F32R, tag=f"E{b}", name=f"E{b}")
                    for b in range(B_LOC)]
            S_sb = persist.tile([128, CC * C], F32, tag="S", name="S")

            # ---- phase 1: transposes + q/k projections ----
            with (
                tc.tile_pool(name="ps_xt", bufs=2, space="PSUM") as ps_xt,
                tc.tile_pool(name="ps_qk", bufs=2, space="PSUM") as ps_qk,
                tc.tile_pool(name="ps_kq", bufs=2, space="PSUM") as ps_kq,
            ):
                for b in range(B_LOC):
                    qk_ps = ps_qk.tile([2 * D, C], F32)
                    for sc in range(SC):
                        xt_ps = ps_xt.tile([128, C], F32)
                        for cc in range(CC):
                            nc.tensor.transpose(
                                xt_ps[:, cc * 128:(cc + 1) * 128],
                                x_sb[b][cc][:, sc * 128:(sc + 1) * 128]
                                .bitcast(F32),
                                ident)
                        xt_sb = xtp.tile([128, C], F32R)
                        if sc % 2 == 0:
                            nc.vector.tensor_copy(xt_sb, xt_ps)
                        else:
                            nc.scalar.copy(xt_sb, xt_ps)
                        nc.tensor.matmul(
                            qk_ps,
                            lhsT=wqk[:, sc, :],
                            rhs=xt_sb,
                            start=(sc == 0), stop=(sc == SC - 1))
                    # bias add, then split k rows down to partition base 0
                    nc.vector.tensor_scalar_add(qkb_sb[b], qk_ps, bqk)
                    nc.sync.dma_start(out=k_sb[b], in_=qkb_sb[b][D:2 * D, :])

                # ---- phase 2: kq + exp ----
                for b in range(B_LOC):
                    for cc in range(CC):
                        kq_ps = ps_kq.tile([128, C], F32)
                        nc.tensor.matmul(
                            kq_ps,
                            lhsT=qkb_sb[b][0:D, cc * 128:(cc + 1) * 128],
                            rhs=k_sb[b],
                            start=True, stop=True)
                        nc.scalar.activation(
                            out=E_sb[b][:, cc * C:(cc + 1) * C], in_=kq_ps,
                            func=mybir.ActivationFunctionType.Exp)

            # ---- phase 3-6: local sum, AllReduce, normalize ----
            nc.vector.tensor_add(S_sb, E_sb[0], E_sb[1])
            cc_in = dram.tile([128, CC * C], F32, tag="cc_in", name="cc_in")
            cc_out = dram.tile([128, CC * C], F32, tag="cc_out", name="cc_out")
            nc.sync.dma_start(out=cc_in, in_=S_sb)
            nc.gpsimd.collective_compute(
                "AllReduce", mybir.AluOpType.add,
                replica_groups=[list(range(N_CORES))],
                ins=[cc_in.opt()], outs=[cc_out.opt()])
            nc.sync.dma_start(out=S_sb, in_=cc_out)
            nc.vector.reciprocal(S_sb, S_sb)
            for b in range(B_LOC):
                nc.vector.tensor_mul(E_sb[b], E_sb[b], S_sb)

            # ---- phase 7: out[b] = A[b].T @ x[b] ----
            with tc.tile_pool(name="ps_out", bufs=8, space="PSUM") as ps_out:
                for b in range(B_LOC):
                    for oc in range(CC):
                        for sg in range(2):
                            outps = [ps_out.tile([128, 512], F32, tag="outps", name=f"outps{j}")
                                     for j in range(4)]
                            for ic in range(CC):
                                for j in range(4):
                                    sc2 = sg * 4 + j
                                    nc.tensor.matmul(
                                        outps[j],
                                        lhsT=E_sb[b][:, ic * C + oc * 128:
                                                     ic * C + oc * 128 + 128],
                                        rhs=x_sb[b][ic][:, sc2 * 512:
                                                        (sc2 + 1) * 512],
                                        start=(ic == 0), stop=(ic == CC - 1))
                            for j in range(4):
                                sc2 = sg * 4 + j
                                o_sb = outp.tile([128, 512], F32)
                                if j % 2 == 0:
                                    nc.vector.tensor_copy(o_sb, outps[j])
                                else:
                                    nc.scalar.copy(o_sb, outps[j])
                                nc.sync.dma_start(
                                    out=out_d.ap()[b,
                                                   oc * 128:(oc + 1) * 128,
                                                   sc2 * 512:(sc2 + 1) * 512],
                                    in_=o_sb)
    nc.compile()
    return nc


def kernel(x, Wq, bq, Wk, bk):
    x = np.ascontiguousarray(x, dtype=np.float32)
    b_, c_, w_, h_ = x.shape
    xf = x.reshape(b_, c_, w_ * h_)
    wqkT = np.ascontiguousarray(
        np.concatenate([Wq, Wk], axis=0).T.astype(np.float32))   # [S, 32]
    bqk = np.concatenate([bq, bk]).astype(np.float32).reshape(2 * D, 1)

    if "nc" not in _CACHE:
        _CACHE["nc"] = _build()
    nc = _CACHE["nc"]

    in_maps = [
        {"x": np.ascontiguousarray(xf[B_LOC * j: B_LOC * (j + 1)]),
         "wqkT": wqkT, "bqk": bqk}
        for j in range(N_CORES)
    ]
    trace = bool(int(os.environ.get("BASSKERNEL_TRACE", "0")))
    res = run_bass_kernel_spmd(nc, in_maps, core_ids=list(range(N_CORES)),
                               trace=trace)
    _CACHE["last_result"] = res
    out = np.concatenate([r["out"] for r in res.results], axis=0)
    return out.reshape(b_, c_, w_, h_)
